# revision 8
# baseline (speedup 1.0000x reference)
"""MiniRocket feature extractor on 8 Trainium2 NeuronCores (optimized).

Per core (4 batch rows), per (dilation, batch) unit:
  - one strided DMA builds xshift [72, 4096] fp32r in SBUF (9 dilated tap
    shifts x 8 channels of the zero-padded series),
  - TensorE computes resp = W^T @ xshift (contraction 72 = channel x tap;
    fp32r streams at full rate),
  - ScalarE drains PSUM fp32 -> SBUF fp16,
  - VectorE runs fused compare+count (is_gt + add-reduce) per bias over the
    full range, plus one strided pass covering both pad edges at once,
  - GpSimd applies the trim-parity/normalization finalize and the output DMA
    (keeps the sync DMA queue free for input prefetch).
"""

import numpy as np
from contextlib import ExitStack

import concourse.bass as bass
import concourse.mybir as mybir
import concourse.tile as tile
from concourse.ap import AP
from concourse.bass_utils import run_bass_kernel_spmd

DILATIONS = (1, 2, 4, 8, 16, 32)
ND = 6
K = 84
KS = 9
C = 8
L = 4096
F = 4
B = 32
N_CORES = 8
B_LOC = 4
PADMAX = 128
LP = L + 2 * PADMAX
KP = 96
NFEAT = ND * K * F
# (dilation, f) full-range counts computed on ScalarE instead of VectorE
ACT_OFFLOAD = frozenset({(5, 0)})

F32 = mybir.dt.float32
F16 = mybir.dt.float16
F32R = mybir.dt.float32r


def _split_excess_waits(nc, max_waits=1):
    """This walrus build allows only one sync-wait per instruction; hoist
    extra waits onto preceding NOPs of the same engine."""
    n = 0
    for f in nc.m.functions:
        for bb in f.blocks:
            insts = bb.instructions
            if not any(
                i.sync_info and i.sync_info.on_wait and len(i.sync_info.on_wait) > max_waits
                for i in insts
            ):
                continue
            out = []
            for inst in insts:
                si = inst.sync_info
                waits = list(si.on_wait) if si and si.on_wait else []
                if len(waits) > max_waits:
                    for w in waits[:-max_waits]:
                        nop = mybir.InstNoOp(name=f"syncfix-{n}", ins=[], outs=[])
                        n += 1
                        nop.engine = inst.engine
                        nop.sync_info = mybir.SyncInfo(on_wait=[w], on_update=[])
                        out.append(nop)
                    inst.sync_info = mybir.SyncInfo(
                        on_wait=waits[-max_waits:],
                        on_update=list(si.on_update or []),
                    )
                out.append(inst)
            bb.instructions = out


def _build_nc():
    nc = bass.Bass()
    xprep = nc.declare_dram_parameter("xprep", [B_LOC, C, LP], F32R, isOutput=False)
    wstack = nc.declare_dram_parameter("wstack", [ND, 72, KP], F32R, isOutput=False)
    biases = nc.declare_dram_parameter("biases", [ND, KP, F], F32, isOutput=False)
    coef_a = nc.declare_dram_parameter("coef_a", [ND, KP, F], F32, isOutput=False)
    coef_b = nc.declare_dram_parameter("coef_b", [ND, KP, F], F32, isOutput=False)
    nmean = nc.declare_dram_parameter("nmean", [ND, KP, F], F32, isOutput=False)
    nstd = nc.declare_dram_parameter("nstd", [ND, KP, F], F32, isOutput=False)
    out = nc.declare_dram_parameter("out", [B_LOC, NFEAT], F32, isOutput=True)

    alu = mybir.AluOpType
    NCC = ND * F

    with tile.TileContext(nc) as tc, ExitStack() as ctx:
        cpool = ctx.enter_context(tc.tile_pool(name="const", bufs=1))
        xsh_pool = ctx.enter_context(tc.tile_pool(name="xsh", bufs=8))
        psum_pool = ctx.enter_context(tc.tile_pool(name="psum", bufs=2, space="PSUM"))
        resp_pool = ctx.enter_context(tc.tile_pool(name="resp", bufs=4))
        trash_pool = ctx.enter_context(tc.tile_pool(name="trash", bufs=1))
        acc_pool = ctx.enter_context(tc.tile_pool(name="acc", bufs=8))
        feat_pool = ctx.enter_context(tc.tile_pool(name="feat", bufs=12))

        w_t = cpool.tile([72, ND * KP], F32R)
        nc.sync.dma_start(w_t[:], AP(wstack, 0, [[KP, 72], [72 * KP, ND], [1, KP]]))

        def load_c(dram, tag):
            t = cpool.tile([KP, NCC], F32, tag=tag)
            nc.sync.dma_start(t[:], AP(dram, 0, [[F, KP], [KP * F, ND], [1, F]]))
            return t

        bias_t = load_c(biases, "bias_t")
        a_base = load_c(coef_a, "a_base")
        b_base = load_c(coef_b, "b_base")
        mean_t = load_c(nmean, "mean_t")
        std_t = load_c(nstd, "std_t")

        rstd_t = cpool.tile([KP, NCC], F32, tag="rstd_t")
        nc.vector.reciprocal(rstd_t[:], std_t[:])
        a_t = cpool.tile([KP, NCC], F32, tag="a_t")
        nc.gpsimd.tensor_mul(a_t[:], a_base[:], rstd_t[:])
        b_t = cpool.tile([KP, NCC], F32, tag="b_t")
        nc.gpsimd.tensor_mul(b_t[:], b_base[:], rstd_t[:])
        c_t = cpool.tile([KP, NCC], F32, tag="c_t")
        nc.gpsimd.tensor_mul(c_t[:], mean_t[:], rstd_t[:])
        nbias_t = cpool.tile([KP, NCC], F32, tag="nbias_t")
        nc.gpsimd.tensor_scalar_mul(nbias_t[:], bias_t[:], -1.0)

        trash = trash_pool.tile([KP, L], F16)
        trash2 = trash_pool.tile([KP, L], F16, tag="trash2")

        for di, d in enumerate(DILATIONS):
            pad = 4 * d
            w_d = w_t[:, di * KP : (di + 1) * KP]
            for b in range(B_LOC):
                xsh = xsh_pool.tile([72, L], F32R)
                nc.sync.dma_start(
                    xsh[:],
                    AP(xprep, b * C * LP + (PADMAX - pad), [[d, KS], [LP, C], [1, L]]),
                )

                resp16 = resp_pool.tile([KP, L], F16)
                for h in range(2):
                    ps = psum_pool.tile([KP, 2048], F32)
                    for n in range(4):
                        nc.tensor.matmul(
                            ps[:, n * 512 : (n + 1) * 512],
                            w_d,
                            xsh[:, h * 2048 + n * 512 : h * 2048 + (n + 1) * 512],
                            start=True,
                            stop=True,
                        )
                    nc.scalar.copy(resp16[:, h * 2048 : (h + 1) * 2048], ps[:])

                acc = acc_pool.tile([KP, 2 * F], F32)
                col0 = di * F
                pstep = resp16[:].ap[0][0]
                tstep = trash[:].ap[0][0]
                for f in range(F):
                    b_ap = bias_t[:, col0 + f : col0 + f + 1]
                    if (di, f) in ACT_OFFLOAD:
                        # full count on ScalarE: sum of sign(resp - bias);
                        # finalize coefs are adjusted host-side
                        nc.scalar.activation(
                            trash2[:], resp16[:],
                            mybir.ActivationFunctionType.Sign,
                            bias=nbias_t[:, col0 + f : col0 + f + 1],
                            accum_out=acc[:, f : f + 1],
                        )
                    else:
                        nc.vector.tensor_scalar(
                            trash[:], resp16[:], b_ap, None, alu.is_gt, alu.add,
                            accum_out=acc[:, f : f + 1],
                        )
                    # both pad edges in one strided op: free dims [2, pad]
                    ein = AP(
                        resp16[:].tensor, resp16[:].offset,
                        [[pstep, KP], [L - pad, 2], [1, pad]],
                    )
                    eout = AP(
                        trash[:].tensor, trash[:].offset,
                        [[tstep, KP], [L - pad, 2], [1, pad]],
                    )
                    nc.vector.tensor_scalar(
                        eout, ein, b_ap, None, alu.is_gt, alu.add,
                        accum_out=acc[:, F + f : F + f + 1],
                    )

                # finalize on GPSIMD: feat = full*A - edges*B - C
                a_d = a_t[:, col0 : col0 + F]
                b_d = b_t[:, col0 : col0 + F]
                c_d = c_t[:, col0 : col0 + F]
                u = feat_pool.tile([KP, F], F32)
                nc.gpsimd.tensor_mul(u[:], acc[:, 0:F], a_d)
                w2 = feat_pool.tile([KP, F], F32)
                nc.gpsimd.tensor_mul(w2[:], acc[:, F : 2 * F], b_d)
                ft = feat_pool.tile([KP, F], F32)
                nc.gpsimd.tensor_sub(ft[:], u[:], w2[:])
                fn = feat_pool.tile([KP, F], F32)
                nc.gpsimd.tensor_sub(fn[:], ft[:], c_d)

                dst = AP(out, b * NFEAT + di * K * F, [[F, K], [1, F]])
                nc.gpsimd.dma_start(dst, fn[0:K, :])

    _split_excess_waits(nc)
    return nc


_NC_CACHE = None


def _get_nc():
    global _NC_CACHE
    if _NC_CACHE is None:
        _NC_CACHE = _build_nc()
    return _NC_CACHE


LAST_RESULTS = None


def kernel(x, channel_masks, bias_matrices, feature_mean, feature_std):
    global LAST_RESULTS
    x = np.ascontiguousarray(np.asarray(x, dtype=np.float32))
    masks = np.asarray(channel_masks, dtype=np.float32)
    biasm = np.asarray(bias_matrices, dtype=np.float32)
    mean = np.asarray(feature_mean, dtype=np.float32)
    std = np.asarray(feature_std, dtype=np.float32)

    wstack = np.zeros((ND, 72, KP), np.float32)
    for di in range(ND):
        wt = -masks[di].T
        for j in range(KS):
            wstack[di, j * C : (j + 1) * C, :K] = wt
    biases_pad = np.full((ND, KP, F), 1e30, np.float32)
    biases_pad[:, :K, :] = biasm

    coef_a = np.zeros((ND, KP, F), np.float32)
    coef_b = np.zeros((ND, KP, F), np.float32)
    nmean = np.zeros((ND, KP, F), np.float32)
    nstd = np.ones((ND, KP, F), np.float32)
    nmean[:, :K, :] = mean.reshape(ND, K, F)
    nstd[:, :K, :] = std.reshape(ND, K, F)
    for di, d in enumerate(DILATIONS):
        pad = 4 * d
        lt = L - 2 * pad
        par = ((di + np.arange(K)) % 2 == 1).astype(np.float32)[:, None]
        coef_a[di, :K, :] = np.where(par > 0, 1.0 / lt, 1.0 / L)
        coef_b[di, :K, :] = np.where(par > 0, 1.0 / lt, 0.0)
    # ACT-offloaded (d, f) pairs accumulate sum(sign(resp-b)) = 2*count - L,
    # so halve A and shift the mean term: feat = sgn*(A/2) - e*B - (C - L*A/2)
    for (di, f) in ACT_OFFLOAD:
        coef_a[di, :, f] *= 0.5
        nmean[di, :, f] -= float(L) * coef_a[di, :, f]

    xt = np.ascontiguousarray(x.transpose(0, 2, 1))
    xp = np.zeros((B, C, LP), np.float32)
    xp[:, :, PADMAX : PADMAX + L] = xt

    nc = _get_nc()
    in_maps = []
    for core in range(N_CORES):
        in_maps.append(
            {
                "xprep": np.ascontiguousarray(xp[core * B_LOC : (core + 1) * B_LOC]),
                "wstack": wstack,
                "biases": biases_pad,
                "coef_a": coef_a,
                "coef_b": coef_b,
                "nmean": nmean,
                "nstd": nstd,
            }
        )
    res = run_bass_kernel_spmd(nc, in_maps, list(range(N_CORES)))
    LAST_RESULTS = res
    out = np.concatenate([res.results[i]["out"] for i in range(N_CORES)], axis=0)
    return out.astype(np.float32)
